# revision 14
# baseline (speedup 1.0000x reference)
"""Trainium2 Bass kernel for nn_MinibatchDiscrimination — v7.

Reference computation (N=256, A=1024, B=128, C=32):
    M  = einsum('na,abc->nbc', x, T)                      # (N,B,C)
    l1 = sum_c |M[n,b,c] - M[m,b,c]|                      # (N,N,B)
    o  = sum_m exp(-l1)                                   # (N,B)
    out = concat([x, o], axis=1)                          # (N, A+B)

Sharding: B split across 8 cores, 16 kernels each.

v8 = baseline strip structure (all bf16, no fp8 DoubleRow — fp8 matmuls
trigger the heavier DVFS activity-1 throttle) plus:
  - quadrant symmetry: the relu/D free dim is [beta0: m 0..256 |
    beta1: m 128..256] = 384 cols; the (n>=128, m<128) quadrant is
    recovered via column sums of exp tile ed0[:,128:256] (one PE matmul
    per kernel, lhsT = ed slice, rhs = ones column) + a DVE add.
  - relu halves cycle over engines DVE:ACT = 5:3 per half.
  - input DMAs split across the Sync and GPSIMD DGE queues.
  - prologue (R/Bias/NBias/S) emitted one kernel ahead; epilogue
    (exp/csum/add) one kernel behind, so neither engine stalls at
    kernel boundaries.

Pairwise L1 uses the relu + rank-1 identity:
    sum_c |d_c| = 2*sum_c relu(d_c) - S[m] + S[n],  d = M[m,:] - M[n,:].
Both rank-1 terms use the same bf16-rounded S values so the diagonal
cancels exactly and exp(0)=1 dominates o with full fp32 accuracy.
"""

from contextlib import ExitStack

import numpy as np
import ml_dtypes

import concourse.bass as bass
import concourse.bacc as bacc
import concourse.tile as tile
from concourse import mybir
from concourse.bass_utils import run_bass_kernel_spmd

N, A, B, C = 256, 1024, 128, 32
NCORES = 8
BLOC = B // NCORES            # 16 kernels per core
BC = BLOC * C                 # 512 = (b,c) pairs per core
KT = A // 128                 # 8 contraction tiles
NQ = 64                       # quads per kernel b (4 samples each)
FD = 384                      # relu/D free dim: 256 (beta0) + 128 (beta1)

F32 = mybir.dt.float32
BF16 = mybir.dt.bfloat16
ALU = mybir.AluOpType
ACTF = mybir.ActivationFunctionType

_bf = ml_dtypes.bfloat16

QUAD_CYCLE = ["v", "v", "a", "v", "v", "a", "v", "a"]


def _build_twos8() -> np.ndarray:
    """bf16 lhsT bank: variant j (cols 32j..32j+32) has 2.0 at [g*32+c, 4j+g]."""
    w = np.zeros((128, 256), np.float32)
    for j in range(8):
        for g in range(4):
            w[g * 32:(g + 1) * 32, 32 * j + 4 * j + g] = 2.0
    return w.astype(_bf)


def build_nc():
    nc = bacc.Bacc("TRN2", target_bir_lowering=False, debug=False)

    xT_d = nc.declare_dram_parameter("xT", [A, N], BF16, isOutput=False)
    Tl_d = nc.declare_dram_parameter("Tl", [A, BC], BF16, isOutput=False)
    twos_d = nc.declare_dram_parameter("twos8", [128, 256], BF16, isOutput=False)
    onecol_d = nc.declare_dram_parameter("onecol", [128, 1], BF16, isOutput=False)
    onerow_d = nc.declare_dram_parameter("onerow", [1, N], BF16, isOutput=False)
    minrow_d = nc.declare_dram_parameter("minrow", [1, 128], BF16, isOutput=False)
    o_d = nc.declare_dram_parameter("o_raw", [2, 128, BLOC], F32, isOutput=True)

    xT = xT_d.ap()
    Tl = Tl_d.ap()
    o_out = o_d.ap()

    with tile.TileContext(nc) as tc, ExitStack() as ctx:
        singles = ctx.enter_context(tc.tile_pool(name="singles", bufs=1))

        twos_sb = singles.tile([128, 256], BF16, tag="twos8")
        nc.sync.dma_start(out=twos_sb[:], in_=twos_d.ap()[:, :])
        onecol_sb = singles.tile([128, 1], BF16, tag="onecol")
        nc.sync.dma_start(out=onecol_sb[:], in_=onecol_d.ap()[:, :])
        onerow_sb = singles.tile([1, N], BF16, tag="onerow")
        nc.sync.dma_start(out=onerow_sb[:], in_=onerow_d.ap()[:, :])
        minrow_sb = singles.tile([1, 128], BF16, tag="minrow")
        nc.sync.dma_start(out=minrow_sb[:], in_=minrow_d.ap()[:, :])

        xT_sb = []
        Tl_sb = []
        for k in range(KT):
            xk = singles.tile([128, N], BF16, tag=f"xT{k}")
            nc.gpsimd.dma_start(out=xk[:], in_=xT[k * 128:(k + 1) * 128, :])
            xT_sb.append(xk)
            tk = singles.tile([128, BC], BF16, tag=f"Tl{k}")
            nc.sync.dma_start(out=tk[:], in_=Tl[k * 128:(k + 1) * 128, :])
            Tl_sb.append(tk)

        mb_sb = []   # bf16 M^T tiles, resident in SBUF
        mf_sb = []   # f32 M^T tiles (same bf16-rounded values), resident

        # ---- phase 1: MT[(b c), n] = sum_a Tl[a, bc] * xT[a, n] ----
        mtps = ctx.enter_context(tc.tile_pool(name="mtps", bufs=2, space="PSUM"))
        for jj in range(BC // 128):
            ps = mtps.tile([128, N], F32, tag="mt")
            for k in range(KT):
                nc.tensor.matmul(
                    ps[:],
                    Tl_sb[k][:, jj * 128:(jj + 1) * 128],
                    xT_sb[k][:],
                    start=(k == 0),
                    stop=(k == KT - 1),
                )
            mb = singles.tile([128, N], BF16, tag=f"mtbf{jj}")
            nc.vector.tensor_copy(mb[:], ps[:])
            mb_sb.append(mb)
            mf = singles.tile([128, N], F32, tag=f"mtf32{jj}")
            nc.scalar.copy(mf[:], mb[:])
            mf_sb.append(mf)

        # ---- phase 2 ----
        o_sb = singles.tile([128, 2 * BLOC], F32, tag="osb")

        rpool = ctx.enter_context(tc.tile_pool(name="rpool", bufs=3))
        nbias = ctx.enter_context(tc.tile_pool(name="nbias", bufs=3))
        rtpool = ctx.enter_context(tc.tile_pool(name="rtpool", bufs=14))
        srowp = ctx.enter_context(tc.tile_pool(name="srowp", bufs=3))
        edump = ctx.enter_context(tc.tile_pool(name="edump", bufs=3))
        dpool = ctx.enter_context(tc.tile_pool(name="dpool", bufs=3, space="PSUM"))
        auxps = ctx.enter_context(tc.tile_pool(name="auxps", bufs=1, space="PSUM"))
        csump = ctx.enter_context(tc.tile_pool(name="csump", bufs=1, space="PSUM"))

        # beta-half (offset, width) of the rt/D free dim
        SLC = [(0, 256), (256, 128)]

        def prologue(b):
            jj, prow = b // 4, (b % 4) * 32
            R = rpool.tile([128, N], BF16, tag="R")
            NBias = nbias.tile([128, NQ], F32, tag="NBias")
            for g in range(4):
                nc.vector.tensor_copy(
                    R[g * 32:(g + 1) * 32, :],
                    mb_sb[jj][prow:prow + 32, :])
                # NBias[g*32+c, q] = -MT[b*32+c, 4q+g]
                src = mf_sb[jj][prow:prow + 32, :].rearrange(
                    "c (q g) -> c g q", g=4)[:, g, :]
                nc.vector.tensor_scalar_mul(
                    NBias[g * 32:(g + 1) * 32, :], src, -1.0)

            srow_ps = auxps.tile([1, N], F32, tag="srow")
            nc.tensor.matmul(
                srow_ps[:],
                onecol_sb[prow:prow + 32, 0:1],
                mb_sb[jj][prow:prow + 32, :],
                start=True, stop=True,
                tile_position=(prow, 0))
            posS = srowp.tile([1, N], BF16, tag="posS")
            nc.vector.tensor_copy(posS[:], srow_ps[:])
            return R, NBias, posS

        def emit_epilogue(D, b):
            # exp: beta0 full row sums; beta1 partial (m>=128)
            ed0 = edump.tile([128, N], BF16, tag="ed0")
            nc.scalar.activation(
                out=ed0[:], in_=D[:, 0:256],
                func=ACTF.Exp, scale=-1.0,
                accum_out=o_sb[:, b:b + 1])
            ed1 = edump.tile([128, 128], BF16, tag="ed1")
            nc.scalar.activation(
                out=ed1[:], in_=D[:, 256:384],
                func=ACTF.Exp, scale=-1.0,
                accum_out=o_sb[:, BLOC + b:BLOC + b + 1])
            # missing (n>=128, m<128) block by symmetry: column sums of
            # ed0[:, 128:256] -> (128,1), added into o_sb[:, BLOC+b]
            csum = csump.tile([128, 1], F32, tag="csum")
            nc.tensor.matmul(
                csum[:], ed0[:, 128:256], onecol_sb[:, 0:1],
                start=True, stop=True, skip_group_check=True)
            nc.vector.tensor_tensor(
                o_sb[:, BLOC + b:BLOC + b + 1],
                o_sb[:, BLOC + b:BLOC + b + 1],
                csum[:], ALU.add)

        pro = prologue(0)
        pend = None
        qctr = 0
        for b in range(BLOC):
            R, NBias, posS = pro
            if b + 1 < BLOC:
                pro = prologue(b + 1)

            D = dpool.tile([128, FD], F32, tag="D")

            # i-loop: strips interleaved (s fastest) as in the baseline
            for i in range(32):
                s, j = i % 4, i // 4
                qh = 8 * s + j
                rt = rtpool.tile([128, FD], BF16, tag="rt")
                for beta in range(2):
                    off, w = SLC[beta]
                    q = beta * 32 + qh
                    eng = QUAD_CYCLE[qctr % len(QUAD_CYCLE)]
                    qctr += 1
                    if eng == "a":
                        nc.scalar.activation(
                            out=rt[:, off:off + w],
                            in_=R[:, 128 * beta:128 * beta + w],
                            func=ACTF.Relu,
                            bias=NBias[:, q:q + 1], scale=1.0)
                    else:
                        nc.vector.tensor_scalar(
                            rt[:, off:off + w],
                            R[:, 128 * beta:128 * beta + w],
                            NBias[:, q:q + 1], 0.0,
                            ALU.add, ALU.max)
                nc.tensor.matmul(
                    D[32 * s:32 * s + 32, :],
                    twos_sb[:, 32 * j:32 * j + 32],
                    rt[:],
                    start=(j == 0),
                    stop=False,
                    tile_position=(0, 32 * s),
                    skip_group_check=True)

            # rank-1 corrections: D += -S[m] (free) + S[n] (partition)
            for beta in range(2):
                off, w = SLC[beta]
                nc.tensor.matmul(
                    D[:, off:off + w], minrow_sb[:, 0:128],
                    posS[0:1, 128 * beta:128 * beta + w],
                    start=False, stop=False, skip_group_check=True)
            for beta in range(2):
                off, w = SLC[beta]
                nc.tensor.matmul(
                    D[:, off:off + w],
                    posS[:, beta * 128:(beta + 1) * 128],
                    onerow_sb[:, 0:w],
                    start=False, stop=(beta == 1), skip_group_check=True)

            if pend is not None:
                emit_epilogue(*pend)
            pend = (D, b)

        emit_epilogue(*pend)

        for beta in range(2):
            nc.sync.dma_start(
                out=o_out[beta],
                in_=o_sb[:, beta * BLOC:(beta + 1) * BLOC])

    nc.compile()
    return nc


_NC = None


def _get_nc():
    global _NC
    if _NC is None:
        _NC = build_nc()
    return _NC


def _prep_inputs(x: np.ndarray, T: np.ndarray):
    xT_bf = np.ascontiguousarray(x.T).astype(_bf)
    twos8 = _build_twos8()
    onecol = np.ones((128, 1), np.float32).astype(_bf)
    onerow = np.ones((1, N), np.float32).astype(_bf)
    minrow = (-np.ones((1, 128), np.float32)).astype(_bf)
    in_maps = []
    for core in range(NCORES):
        Tl = np.ascontiguousarray(
            T[:, core * BLOC:(core + 1) * BLOC, :].reshape(A, BC)).astype(_bf)
        in_maps.append({"xT": xT_bf, "Tl": Tl, "twos8": twos8,
                        "onecol": onecol, "onerow": onerow,
                        "minrow": minrow})
    return in_maps


def _assemble(x: np.ndarray, results) -> np.ndarray:
    o = np.zeros((N, B), np.float32)
    for core in range(NCORES):
        o_raw = results[core]["o_raw"]          # (2, 128, BLOC) f32
        o[:128, core * BLOC:(core + 1) * BLOC] = o_raw[0]
        o[128:, core * BLOC:(core + 1) * BLOC] = o_raw[1]
    return np.concatenate([x.astype(np.float32), o], axis=1)


def run_device(x: np.ndarray, T: np.ndarray, trace: bool = False):
    """Run the SPMD kernel; returns (full output, BassKernelResults)."""
    nc = _get_nc()
    in_maps = _prep_inputs(x, T)
    res = run_bass_kernel_spmd(nc, in_maps, list(range(NCORES)), trace=trace)
    return _assemble(x, res.results), res


def kernel(x: np.ndarray, T: np.ndarray) -> np.ndarray:
    x = np.asarray(x, dtype=np.float32)
    T = np.asarray(T, dtype=np.float32)
    out, _ = run_device(x, T)
    return out


if __name__ == "__main__":
    rng = np.random.default_rng(0)
    x = rng.standard_normal((N, A)).astype(np.float32)
    T = (rng.standard_normal((A, B, C)) * 0.05).astype(np.float32)
    out = kernel(x, T)
    print("out", out.shape, out.dtype)


# revision 15
# speedup vs baseline: 1.1555x; 1.1555x over previous
"""Trainium2 Bass kernel for nn_MinibatchDiscrimination — v7.

Reference computation (N=256, A=1024, B=128, C=32):
    M  = einsum('na,abc->nbc', x, T)                      # (N,B,C)
    l1 = sum_c |M[n,b,c] - M[m,b,c]|                      # (N,N,B)
    o  = sum_m exp(-l1)                                   # (N,B)
    out = concat([x, o], axis=1)                          # (N, A+B)

Sharding: B split across 8 cores, 16 kernels each.

v8 = baseline strip structure (all bf16, no fp8 DoubleRow — fp8 matmuls
trigger the heavier DVFS activity-1 throttle) plus:
  - quadrant symmetry: the relu/D free dim is [beta0: m 0..256 |
    beta1: m 128..256] = 384 cols; the (n>=128, m<128) quadrant is
    recovered via column sums of exp tile ed0[:,128:256] (one PE matmul
    per kernel, lhsT = ed slice, rhs = ones column) + a DVE add.
  - relu halves cycle over engines DVE:ACT = 5:3 per half.
  - input DMAs split across the Sync and GPSIMD DGE queues.
  - prologue (R/Bias/NBias/S) emitted one kernel ahead; epilogue
    (exp/csum/add) one kernel behind, so neither engine stalls at
    kernel boundaries.

Pairwise L1 uses the relu + rank-1 identity:
    sum_c |d_c| = 2*sum_c relu(d_c) - S[m] + S[n],  d = M[m,:] - M[n,:].
Both rank-1 terms use the same bf16-rounded S values so the diagonal
cancels exactly and exp(0)=1 dominates o with full fp32 accuracy.
"""

from contextlib import ExitStack

import numpy as np
import ml_dtypes

import concourse.bass as bass
import concourse.bacc as bacc
import concourse.tile as tile
from concourse import mybir
from concourse.bass_utils import run_bass_kernel_spmd

N, A, B, C = 256, 1024, 128, 32
NCORES = 8
BLOC = B // NCORES            # 16 kernels per core
BC = BLOC * C                 # 512 = (b,c) pairs per core
KT = A // 128                 # 8 contraction tiles
NQ = 64                       # quads per kernel b (4 samples each)
FD = 384                      # relu/D free dim: 256 (beta0) + 128 (beta1)

F32 = mybir.dt.float32
BF16 = mybir.dt.bfloat16
ALU = mybir.AluOpType
ACTF = mybir.ActivationFunctionType

_bf = ml_dtypes.bfloat16

QUAD_CYCLE = ["v", "v", "a", "v", "v", "a", "v", "a"]


def _build_twos8() -> np.ndarray:
    """bf16 lhsT bank: variant j (cols 32j..32j+32) has 2.0 at [g*32+c, 4j+g]."""
    w = np.zeros((128, 256), np.float32)
    for j in range(8):
        for g in range(4):
            w[g * 32:(g + 1) * 32, 32 * j + 4 * j + g] = 2.0
    return w.astype(_bf)


def build_nc():
    nc = bacc.Bacc("TRN2", target_bir_lowering=False, debug=False)

    xT_d = nc.declare_dram_parameter("xT", [A, N], BF16, isOutput=False)
    Tl_d = nc.declare_dram_parameter("Tl", [A, BC], BF16, isOutput=False)
    twos_d = nc.declare_dram_parameter("twos8", [128, 256], BF16, isOutput=False)
    onecol_d = nc.declare_dram_parameter("onecol", [128, 1], BF16, isOutput=False)
    onerow_d = nc.declare_dram_parameter("onerow", [1, N], BF16, isOutput=False)
    minrow_d = nc.declare_dram_parameter("minrow", [1, 128], BF16, isOutput=False)
    o_d = nc.declare_dram_parameter("o_raw", [2, 128, BLOC], F32, isOutput=True)

    xT = xT_d.ap()
    Tl = Tl_d.ap()
    o_out = o_d.ap()

    with tile.TileContext(nc) as tc, ExitStack() as ctx:
        singles = ctx.enter_context(tc.tile_pool(name="singles", bufs=1))

        twos_sb = singles.tile([128, 256], BF16, tag="twos8")
        nc.sync.dma_start(out=twos_sb[:], in_=twos_d.ap()[:, :])
        onecol_sb = singles.tile([128, 1], BF16, tag="onecol")
        nc.sync.dma_start(out=onecol_sb[:], in_=onecol_d.ap()[:, :])
        onerow_sb = singles.tile([1, N], BF16, tag="onerow")
        nc.sync.dma_start(out=onerow_sb[:], in_=onerow_d.ap()[:, :])
        minrow_sb = singles.tile([1, 128], BF16, tag="minrow")
        nc.sync.dma_start(out=minrow_sb[:], in_=minrow_d.ap()[:, :])

        xT_sb = []
        Tl_sb = []
        for k in range(KT):
            xk = singles.tile([128, N], BF16, tag=f"xT{k}")
            nc.gpsimd.dma_start(out=xk[:], in_=xT[k * 128:(k + 1) * 128, :])
            xT_sb.append(xk)
            tk = singles.tile([128, BC], BF16, tag=f"Tl{k}")
            nc.sync.dma_start(out=tk[:], in_=Tl[k * 128:(k + 1) * 128, :])
            Tl_sb.append(tk)

        mb_sb = []   # bf16 M^T tiles, resident in SBUF
        mf_sb = []   # f32 M^T tiles (same bf16-rounded values), resident

        # ---- phase 1: MT[(b c), n] = sum_a Tl[a, bc] * xT[a, n] ----
        mtps = ctx.enter_context(tc.tile_pool(name="mtps", bufs=2, space="PSUM"))
        for jj in range(BC // 128):
            ps = mtps.tile([128, N], F32, tag="mt")
            for k in range(KT):
                nc.tensor.matmul(
                    ps[:],
                    Tl_sb[k][:, jj * 128:(jj + 1) * 128],
                    xT_sb[k][:],
                    start=(k == 0),
                    stop=(k == KT - 1),
                )
            mb = singles.tile([128, N], BF16, tag=f"mtbf{jj}")
            nc.vector.tensor_copy(mb[:], ps[:])
            mb_sb.append(mb)
            mf = singles.tile([128, N], F32, tag=f"mtf32{jj}")
            nc.scalar.copy(mf[:], mb[:])
            mf_sb.append(mf)

        # ---- phase 2 ----
        o_sb = singles.tile([128, 2 * BLOC], F32, tag="osb")

        rpool = ctx.enter_context(tc.tile_pool(name="rpool", bufs=3))
        biasp = ctx.enter_context(tc.tile_pool(name="biasp", bufs=3))
        nbias = ctx.enter_context(tc.tile_pool(name="nbias", bufs=3))
        rtpool = ctx.enter_context(tc.tile_pool(name="rtpool", bufs=10))
        srowp = ctx.enter_context(tc.tile_pool(name="srowp", bufs=3))
        edump = ctx.enter_context(tc.tile_pool(name="edump", bufs=3))
        dpool = ctx.enter_context(tc.tile_pool(name="dpool", bufs=3, space="PSUM"))
        auxps = ctx.enter_context(tc.tile_pool(name="auxps", bufs=1, space="PSUM"))
        csump = ctx.enter_context(tc.tile_pool(name="csump", bufs=1, space="PSUM"))

        # beta-half (offset, width) of the rt/D free dim
        SLC = [(0, 256), (256, 128)]

        def prologue(b):
            jj, prow = b // 4, (b % 4) * 32
            R = rpool.tile([128, N], BF16, tag="R")
            Bias = biasp.tile([128, NQ], F32, tag="Bias")
            for g in range(4):
                nc.vector.tensor_copy(
                    R[g * 32:(g + 1) * 32, :],
                    mb_sb[jj][prow:prow + 32, :])
                # Bias[g*32+c, q] = MT[b*32+c, 4q+g]
                src = mf_sb[jj][prow:prow + 32, :].rearrange(
                    "c (q g) -> c g q", g=4)[:, g, :]
                nc.vector.tensor_copy(Bias[g * 32:(g + 1) * 32, :], src)
            NBias = nbias.tile([128, NQ], F32, tag="NBias")
            nc.vector.tensor_scalar_mul(NBias[:], Bias[:], -1.0)

            srow_ps = auxps.tile([1, N], F32, tag="srow")
            nc.tensor.matmul(
                srow_ps[:],
                onecol_sb[prow:prow + 32, 0:1],
                mb_sb[jj][prow:prow + 32, :],
                start=True, stop=True,
                tile_position=(prow, 0))
            posS = srowp.tile([1, N], BF16, tag="posS")
            nc.vector.tensor_copy(posS[:], srow_ps[:])
            return R, Bias, NBias, posS

        def emit_epilogue(D, b):
            # exp: beta0 full row sums; beta1 partial (m>=128)
            ed0 = edump.tile([128, N], BF16, tag="ed0")
            nc.scalar.activation(
                out=ed0[:], in_=D[:, 0:256],
                func=ACTF.Exp, scale=-1.0,
                accum_out=o_sb[:, b:b + 1])
            ed1 = edump.tile([128, 128], BF16, tag="ed1")
            nc.scalar.activation(
                out=ed1[:], in_=D[:, 256:384],
                func=ACTF.Exp, scale=-1.0,
                accum_out=o_sb[:, BLOC + b:BLOC + b + 1])
            # missing (n>=128, m<128) block by symmetry: column sums of
            # ed0[:, 128:256] -> (128,1), added into o_sb[:, BLOC+b]
            csum = csump.tile([128, 1], F32, tag="csum")
            nc.tensor.matmul(
                csum[:], ed0[:, 128:256], onecol_sb[:, 0:1],
                start=True, stop=True, skip_group_check=True)
            nc.vector.tensor_tensor(
                o_sb[:, BLOC + b:BLOC + b + 1],
                o_sb[:, BLOC + b:BLOC + b + 1],
                csum[:], ALU.add)

        pro = prologue(0)
        pend = None
        qctr = 0
        for b in range(BLOC):
            R, Bias, NBias, posS = pro
            if b + 1 < BLOC:
                pro = prologue(b + 1)

            D = dpool.tile([128, FD], F32, tag="D")

            # i-loop: strips interleaved (s fastest) as in the baseline
            for i in range(32):
                s, j = i % 4, i // 4
                qh = 8 * s + j
                rt = rtpool.tile([128, FD], BF16, tag="rt")
                for beta in range(2):
                    off, w = SLC[beta]
                    q = beta * 32 + qh
                    eng = QUAD_CYCLE[qctr % len(QUAD_CYCLE)]
                    qctr += 1
                    if eng == "a":
                        nc.scalar.activation(
                            out=rt[:, off:off + w],
                            in_=R[:, 128 * beta:128 * beta + w],
                            func=ACTF.Relu,
                            bias=NBias[:, q:q + 1], scale=1.0)
                    else:
                        nc.vector.tensor_scalar(
                            rt[:, off:off + w],
                            R[:, 128 * beta:128 * beta + w],
                            Bias[:, q:q + 1], 0.0,
                            ALU.subtract, ALU.max)
                nc.tensor.matmul(
                    D[32 * s:32 * s + 32, :],
                    twos_sb[:, 32 * j:32 * j + 32],
                    rt[:],
                    start=(j == 0),
                    stop=False,
                    tile_position=(0, 32 * s),
                    skip_group_check=True)

            # rank-1 corrections: D += -S[m] (free) + S[n] (partition)
            for beta in range(2):
                off, w = SLC[beta]
                nc.tensor.matmul(
                    D[:, off:off + w], minrow_sb[:, 0:128],
                    posS[0:1, 128 * beta:128 * beta + w],
                    start=False, stop=False, skip_group_check=True)
            for beta in range(2):
                off, w = SLC[beta]
                nc.tensor.matmul(
                    D[:, off:off + w],
                    posS[:, beta * 128:(beta + 1) * 128],
                    onerow_sb[:, 0:w],
                    start=False, stop=(beta == 1), skip_group_check=True)

            if pend is not None:
                emit_epilogue(*pend)
            pend = (D, b)

        emit_epilogue(*pend)

        for beta in range(2):
            nc.sync.dma_start(
                out=o_out[beta],
                in_=o_sb[:, beta * BLOC:(beta + 1) * BLOC])

    nc.compile()
    return nc


_NC = None


def _get_nc():
    global _NC
    if _NC is None:
        _NC = build_nc()
    return _NC


def _prep_inputs(x: np.ndarray, T: np.ndarray):
    xT_bf = np.ascontiguousarray(x.T).astype(_bf)
    twos8 = _build_twos8()
    onecol = np.ones((128, 1), np.float32).astype(_bf)
    onerow = np.ones((1, N), np.float32).astype(_bf)
    minrow = (-np.ones((1, 128), np.float32)).astype(_bf)
    in_maps = []
    for core in range(NCORES):
        Tl = np.ascontiguousarray(
            T[:, core * BLOC:(core + 1) * BLOC, :].reshape(A, BC)).astype(_bf)
        in_maps.append({"xT": xT_bf, "Tl": Tl, "twos8": twos8,
                        "onecol": onecol, "onerow": onerow,
                        "minrow": minrow})
    return in_maps


def _assemble(x: np.ndarray, results) -> np.ndarray:
    o = np.zeros((N, B), np.float32)
    for core in range(NCORES):
        o_raw = results[core]["o_raw"]          # (2, 128, BLOC) f32
        o[:128, core * BLOC:(core + 1) * BLOC] = o_raw[0]
        o[128:, core * BLOC:(core + 1) * BLOC] = o_raw[1]
    return np.concatenate([x.astype(np.float32), o], axis=1)


def run_device(x: np.ndarray, T: np.ndarray, trace: bool = False):
    """Run the SPMD kernel; returns (full output, BassKernelResults)."""
    nc = _get_nc()
    in_maps = _prep_inputs(x, T)
    res = run_bass_kernel_spmd(nc, in_maps, list(range(NCORES)), trace=trace)
    return _assemble(x, res.results), res


def kernel(x: np.ndarray, T: np.ndarray) -> np.ndarray:
    x = np.asarray(x, dtype=np.float32)
    T = np.asarray(T, dtype=np.float32)
    out, _ = run_device(x, T)
    return out


if __name__ == "__main__":
    rng = np.random.default_rng(0)
    x = rng.standard_normal((N, A)).astype(np.float32)
    T = (rng.standard_normal((A, B, C)) * 0.05).astype(np.float32)
    out = kernel(x, T)
    print("out", out.shape, out.dtype)


# revision 16
# speedup vs baseline: 1.1647x; 1.0080x over previous
"""Trainium2 Bass kernel for nn_MinibatchDiscrimination — v7.

Reference computation (N=256, A=1024, B=128, C=32):
    M  = einsum('na,abc->nbc', x, T)                      # (N,B,C)
    l1 = sum_c |M[n,b,c] - M[m,b,c]|                      # (N,N,B)
    o  = sum_m exp(-l1)                                   # (N,B)
    out = concat([x, o], axis=1)                          # (N, A+B)

Sharding: B split across 8 cores, 16 kernels each.

v8 = baseline strip structure (all bf16, no fp8 DoubleRow — fp8 matmuls
trigger the heavier DVFS activity-1 throttle) plus:
  - quadrant symmetry: the relu/D free dim is [beta0: m 0..256 |
    beta1: m 128..256] = 384 cols; the (n>=128, m<128) quadrant is
    recovered via column sums of exp tile ed0[:,128:256] (one PE matmul
    per kernel, lhsT = ed slice, rhs = ones column) + a DVE add.
  - relu halves cycle over engines DVE:ACT = 5:3 per half.
  - input DMAs split across the Sync and GPSIMD DGE queues.
  - prologue (R/Bias/NBias/S) emitted one kernel ahead; epilogue
    (exp/csum/add) one kernel behind, so neither engine stalls at
    kernel boundaries.

Pairwise L1 uses the relu + rank-1 identity:
    sum_c |d_c| = 2*sum_c relu(d_c) - S[m] + S[n],  d = M[m,:] - M[n,:].
Both rank-1 terms use the same bf16-rounded S values so the diagonal
cancels exactly and exp(0)=1 dominates o with full fp32 accuracy.
"""

from contextlib import ExitStack

import numpy as np
import ml_dtypes

import concourse.bass as bass
import concourse.bacc as bacc
import concourse.tile as tile
from concourse import mybir
from concourse.bass_utils import run_bass_kernel_spmd

N, A, B, C = 256, 1024, 128, 32
NCORES = 8
BLOC = B // NCORES            # 16 kernels per core
BC = BLOC * C                 # 512 = (b,c) pairs per core
KT = A // 128                 # 8 contraction tiles
NQ = 64                       # quads per kernel b (4 samples each)
FD = 384                      # relu/D free dim: 256 (beta0) + 128 (beta1)

F32 = mybir.dt.float32
BF16 = mybir.dt.bfloat16
ALU = mybir.AluOpType
ACTF = mybir.ActivationFunctionType

_bf = ml_dtypes.bfloat16

QUAD_CYCLE = ["v", "v", "a", "v", "v", "a", "v", "a"]


def _build_twos8() -> np.ndarray:
    """bf16 lhsT bank: variant j (cols 32j..32j+32) has 2.0 at [g*32+c, 4j+g]."""
    w = np.zeros((128, 256), np.float32)
    for j in range(8):
        for g in range(4):
            w[g * 32:(g + 1) * 32, 32 * j + 4 * j + g] = 2.0
    return w.astype(_bf)


def build_nc():
    nc = bacc.Bacc("TRN2", target_bir_lowering=False, debug=False)

    xT_d = nc.declare_dram_parameter("xT", [A, N], BF16, isOutput=False)
    Tl_d = nc.declare_dram_parameter("Tl", [A, BC], BF16, isOutput=False)
    twos_d = nc.declare_dram_parameter("twos8", [128, 256], BF16, isOutput=False)
    onecol_d = nc.declare_dram_parameter("onecol", [128, 1], BF16, isOutput=False)
    onerow_d = nc.declare_dram_parameter("onerow", [1, N], BF16, isOutput=False)
    minrow_d = nc.declare_dram_parameter("minrow", [1, 128], BF16, isOutput=False)
    o_d = nc.declare_dram_parameter("o_raw", [2, 128, BLOC], F32, isOutput=True)

    xT = xT_d.ap()
    Tl = Tl_d.ap()
    o_out = o_d.ap()

    with tile.TileContext(nc) as tc, ExitStack() as ctx:
        singles = ctx.enter_context(tc.tile_pool(name="singles", bufs=1))

        twos_sb = singles.tile([128, 256], BF16, tag="twos8")
        nc.sync.dma_start(out=twos_sb[:], in_=twos_d.ap()[:, :])
        onecol_sb = singles.tile([128, 1], BF16, tag="onecol")
        nc.sync.dma_start(out=onecol_sb[:], in_=onecol_d.ap()[:, :])
        onerow_sb = singles.tile([1, N], BF16, tag="onerow")
        nc.sync.dma_start(out=onerow_sb[:], in_=onerow_d.ap()[:, :])
        minrow_sb = singles.tile([1, 128], BF16, tag="minrow")
        nc.sync.dma_start(out=minrow_sb[:], in_=minrow_d.ap()[:, :])

        xT_sb = []
        Tl_sb = []
        for k in range(KT):
            xk = singles.tile([128, N], BF16, tag=f"xT{k}")
            nc.gpsimd.dma_start(out=xk[:], in_=xT[k * 128:(k + 1) * 128, :])
            xT_sb.append(xk)
            tk = singles.tile([128, BC], BF16, tag=f"Tl{k}")
            nc.sync.dma_start(out=tk[:], in_=Tl[k * 128:(k + 1) * 128, :])
            Tl_sb.append(tk)

        mb_sb = {}   # bf16 M^T tiles, resident in SBUF
        mf_sb = {}   # f32 M^T tiles (same bf16-rounded values), resident

        # ---- phase 1: MT[(b c), n] = sum_a Tl[a, bc] * xT[a, n] ----
        mtps = ctx.enter_context(tc.tile_pool(name="mtps", bufs=2, space="PSUM"))
        def phase1_emit(jj):
            ps = mtps.tile([128, N], F32, tag="mt")
            for k in range(KT):
                nc.tensor.matmul(
                    ps[:],
                    Tl_sb[k][:, jj * 128:(jj + 1) * 128],
                    xT_sb[k][:],
                    start=(k == 0),
                    stop=(k == KT - 1),
                )
            mb = singles.tile([128, N], BF16, tag=f"mtbf{jj}")
            nc.vector.tensor_copy(mb[:], ps[:])
            mb_sb[jj] = mb
            mf = singles.tile([128, N], F32, tag=f"mtf32{jj}")
            nc.scalar.copy(mf[:], mb[:])
            mf_sb[jj] = mf

        # ---- phase 2 ----
        o_sb = singles.tile([128, 2 * BLOC], F32, tag="osb")

        rpool = ctx.enter_context(tc.tile_pool(name="rpool", bufs=3))
        biasp = ctx.enter_context(tc.tile_pool(name="biasp", bufs=3))
        nbias = ctx.enter_context(tc.tile_pool(name="nbias", bufs=3))
        rtpool = ctx.enter_context(tc.tile_pool(name="rtpool", bufs=10))
        srowp = ctx.enter_context(tc.tile_pool(name="srowp", bufs=3))
        edump = ctx.enter_context(tc.tile_pool(name="edump", bufs=3))
        dpool = ctx.enter_context(tc.tile_pool(name="dpool", bufs=3, space="PSUM"))
        auxps = ctx.enter_context(tc.tile_pool(name="auxps", bufs=1, space="PSUM"))
        csump = ctx.enter_context(tc.tile_pool(name="csump", bufs=1, space="PSUM"))

        # beta-half (offset, width) of the rt/D free dim
        SLC = [(0, 256), (256, 128)]

        def prologue(b):
            jj, prow = b // 4, (b % 4) * 32
            R = rpool.tile([128, N], BF16, tag="R")
            Bias = biasp.tile([128, NQ], F32, tag="Bias")
            for g in range(4):
                nc.vector.tensor_copy(
                    R[g * 32:(g + 1) * 32, :],
                    mb_sb[jj][prow:prow + 32, :])
                # Bias[g*32+c, q] = MT[b*32+c, 4q+g]
                src = mf_sb[jj][prow:prow + 32, :].rearrange(
                    "c (q g) -> c g q", g=4)[:, g, :]
                nc.vector.tensor_copy(Bias[g * 32:(g + 1) * 32, :], src)
            NBias = nbias.tile([128, NQ], F32, tag="NBias")
            nc.vector.tensor_scalar_mul(NBias[:], Bias[:], -1.0)

            srow_ps = auxps.tile([1, N], F32, tag="srow")
            nc.tensor.matmul(
                srow_ps[:],
                onecol_sb[prow:prow + 32, 0:1],
                mb_sb[jj][prow:prow + 32, :],
                start=True, stop=True,
                tile_position=(prow, 0))
            posS = srowp.tile([1, N], BF16, tag="posS")
            nc.vector.tensor_copy(posS[:], srow_ps[:])
            return R, Bias, NBias, posS

        def emit_epilogue(D, b):
            # exp: beta0 full row sums; beta1 partial (m>=128)
            ed0 = edump.tile([128, N], BF16, tag="ed0")
            nc.scalar.activation(
                out=ed0[:], in_=D[:, 0:256],
                func=ACTF.Exp, scale=-1.0,
                accum_out=o_sb[:, b:b + 1])
            ed1 = edump.tile([128, 128], BF16, tag="ed1")
            nc.scalar.activation(
                out=ed1[:], in_=D[:, 256:384],
                func=ACTF.Exp, scale=-1.0,
                accum_out=o_sb[:, BLOC + b:BLOC + b + 1])
            # missing (n>=128, m<128) block by symmetry: column sums of
            # ed0[:, 128:256] -> (128,1), added into o_sb[:, BLOC+b]
            csum = csump.tile([128, 1], F32, tag="csum")
            nc.tensor.matmul(
                csum[:], ed0[:, 128:256], onecol_sb[:, 0:1],
                start=True, stop=True, skip_group_check=True)
            nc.vector.tensor_tensor(
                o_sb[:, BLOC + b:BLOC + b + 1],
                o_sb[:, BLOC + b:BLOC + b + 1],
                csum[:], ALU.add)

        phase1_emit(0)
        pro = prologue(0)
        pend = None
        qctr = 0
        for b in range(BLOC):
            R, Bias, NBias, posS = pro
            if b % 4 == 2 and b // 4 + 1 < BC // 128:
                phase1_emit(b // 4 + 1)
            if b + 1 < BLOC:
                pro = prologue(b + 1)

            D = dpool.tile([128, FD], F32, tag="D")

            # i-loop: strips interleaved (s fastest) as in the baseline
            for i in range(32):
                s, j = i % 4, i // 4
                qh = 8 * s + j
                rt = rtpool.tile([128, FD], BF16, tag="rt")
                for beta in range(2):
                    off, w = SLC[beta]
                    q = beta * 32 + qh
                    eng = QUAD_CYCLE[qctr % len(QUAD_CYCLE)]
                    qctr += 1
                    if eng == "a":
                        nc.scalar.activation(
                            out=rt[:, off:off + w],
                            in_=R[:, 128 * beta:128 * beta + w],
                            func=ACTF.Relu,
                            bias=NBias[:, q:q + 1], scale=1.0)
                    else:
                        nc.vector.tensor_scalar(
                            rt[:, off:off + w],
                            R[:, 128 * beta:128 * beta + w],
                            Bias[:, q:q + 1], 0.0,
                            ALU.subtract, ALU.max)
                nc.tensor.matmul(
                    D[32 * s:32 * s + 32, :],
                    twos_sb[:, 32 * j:32 * j + 32],
                    rt[:],
                    start=(j == 0),
                    stop=False,
                    tile_position=(0, 32 * s),
                    skip_group_check=True)

            # rank-1 corrections: D += -S[m] (free) + S[n] (partition)
            for beta in range(2):
                off, w = SLC[beta]
                nc.tensor.matmul(
                    D[:, off:off + w], minrow_sb[:, 0:128],
                    posS[0:1, 128 * beta:128 * beta + w],
                    start=False, stop=False, skip_group_check=True)
            for beta in range(2):
                off, w = SLC[beta]
                nc.tensor.matmul(
                    D[:, off:off + w],
                    posS[:, beta * 128:(beta + 1) * 128],
                    onerow_sb[:, 0:w],
                    start=False, stop=(beta == 1), skip_group_check=True)

            if pend is not None:
                emit_epilogue(*pend)
            pend = (D, b)

        emit_epilogue(*pend)

        for beta in range(2):
            nc.sync.dma_start(
                out=o_out[beta],
                in_=o_sb[:, beta * BLOC:(beta + 1) * BLOC])

    nc.compile()
    return nc


_NC = None


def _get_nc():
    global _NC
    if _NC is None:
        _NC = build_nc()
    return _NC


def _prep_inputs(x: np.ndarray, T: np.ndarray):
    xT_bf = np.ascontiguousarray(x.T).astype(_bf)
    twos8 = _build_twos8()
    onecol = np.ones((128, 1), np.float32).astype(_bf)
    onerow = np.ones((1, N), np.float32).astype(_bf)
    minrow = (-np.ones((1, 128), np.float32)).astype(_bf)
    in_maps = []
    for core in range(NCORES):
        Tl = np.ascontiguousarray(
            T[:, core * BLOC:(core + 1) * BLOC, :].reshape(A, BC)).astype(_bf)
        in_maps.append({"xT": xT_bf, "Tl": Tl, "twos8": twos8,
                        "onecol": onecol, "onerow": onerow,
                        "minrow": minrow})
    return in_maps


def _assemble(x: np.ndarray, results) -> np.ndarray:
    o = np.zeros((N, B), np.float32)
    for core in range(NCORES):
        o_raw = results[core]["o_raw"]          # (2, 128, BLOC) f32
        o[:128, core * BLOC:(core + 1) * BLOC] = o_raw[0]
        o[128:, core * BLOC:(core + 1) * BLOC] = o_raw[1]
    return np.concatenate([x.astype(np.float32), o], axis=1)


def run_device(x: np.ndarray, T: np.ndarray, trace: bool = False):
    """Run the SPMD kernel; returns (full output, BassKernelResults)."""
    nc = _get_nc()
    in_maps = _prep_inputs(x, T)
    res = run_bass_kernel_spmd(nc, in_maps, list(range(NCORES)), trace=trace)
    return _assemble(x, res.results), res


def kernel(x: np.ndarray, T: np.ndarray) -> np.ndarray:
    x = np.asarray(x, dtype=np.float32)
    T = np.asarray(T, dtype=np.float32)
    out, _ = run_device(x, T)
    return out


if __name__ == "__main__":
    rng = np.random.default_rng(0)
    x = rng.standard_normal((N, A)).astype(np.float32)
    T = (rng.standard_normal((A, B, C)) * 0.05).astype(np.float32)
    out = kernel(x, T)
    print("out", out.shape, out.dtype)
